# revision 7
# baseline (speedup 1.0000x reference)
"""Distributed ISTFT kernel for Trainium2 (8 NeuronCores, Bass/Tile).

Math (matches the jax reference):
  z: [2, 513, T] one-sided spectrum (real/imag), T = 8192 frames.
  Hermitian extension + ifft(1024) + window + overlap-add (hop 256) +
  divide by overlapped window sum + trim 512 each side -> [2, 2096896].

Key folds used here:
  * real(ifft) = A^T @ X where A [1024(k), 1024(n)] packs the cos rows for
    zr bins 0..512 and sin rows for zi bins 1..511; X packs those z rows.
  * imag(ifft)[n, t] = (zi[0,t] + (-1)^n zi[512,t]) / N  (rank-2).
  * Output sample m = 256*b + r; block b = sum_{q=0..3} wf_{b-q}[256q+r].
    Folding window * A and the reciprocal window-sum into the stationary
    operand gives O^T[t, r] = sum_q X[:, t+3-q]^T @ Aw_q directly -- the
    overlap-add, windowing and normalization all ride inside the matmul.
  * Frame axis is sharded 1024 output blocks/core with a 3-frame input
    halo, so no cross-core communication is needed at all.  The two
    blocks whose window-sum misses a frame (global block 2 and 8192) get
    a data-driven row fixup (masks make the same program a no-op on the
    other cores).
"""

import numpy as np

N_FFT = 1024
HOP = 256
T_FRAMES = 8192
N_CORES = 8
F_SLOTS = 1027  # frame slots per core: 1024 owned blocks need slots t..t+3
NB = 1024       # output blocks computed per core (core 7 uses 1023)

_CACHE = {}


def _amat() -> np.ndarray:
    """A [1024(kappa), 1024(n)]: ifft cos/sin weights, f32."""
    n = np.arange(N_FFT, dtype=np.float64)[None, :]
    k = np.arange(513, dtype=np.float64)[:, None]
    g = np.full((513, 1), 2.0)
    g[0, 0] = 1.0
    g[512, 0] = 1.0
    C = (g / N_FFT) * np.cos(2.0 * np.pi * k * n / N_FFT)
    k2 = np.arange(1, 512, dtype=np.float64)[:, None]
    S = (-2.0 / N_FFT) * np.sin(2.0 * np.pi * k2 * n / N_FFT)
    return np.ascontiguousarray(np.concatenate([C, S], 0).astype(np.float32))


def _consts() -> np.ndarray:
    c = np.zeros((8, 256), np.float32)
    c[0:4, :] = (1.0 - 2.0 * (np.arange(256) % 2)).astype(np.float32)[None, :]
    c[4:8, :] = 1.0
    return c


def _build_nc():
    from contextlib import ExitStack

    import concourse.tile as tile
    from concourse import bacc, mybir

    f32 = mybir.dt.float32
    f32r = mybir.dt.float32r

    nc = bacc.Bacc("TRN2", target_bir_lowering=False, debug=False,
                   num_devices=N_CORES)

    x_d = nc.dram_tensor("x", [1026, F_SLOTS], f32, kind="ExternalInput")
    a_d = nc.dram_tensor("amat", [1024, 1024], f32, kind="ExternalInput")
    w_d = nc.dram_tensor("wvec", [4, 256], f32, kind="ExternalInput")
    c_d = nc.dram_tensor("consts", [8, 256], f32, kind="ExternalInput")
    m_d = nc.dram_tensor("masks", [1, 2], f32, kind="ExternalInput")
    o_d = nc.dram_tensor("out", [2, NB, 256], f32, kind="ExternalOutput")

    with tile.TileContext(nc) as tc, ExitStack() as ctx:
        big = ctx.enter_context(tc.tile_pool(name="big", bufs=1))
        stg = ctx.enter_context(tc.tile_pool(name="stg", bufs=3))
        sml = ctx.enter_context(tc.tile_pool(name="sml", bufs=1))
        pso = ctx.enter_context(tc.tile_pool(name="pso", bufs=3, space="PSUM"))
        osb = ctx.enter_context(tc.tile_pool(name="osb", bufs=4))
        drm = ctx.enter_context(tc.tile_pool(name="drm", bufs=1, space="DRAM"))

        # ---- big loads start first so DMA overlaps the setup chain ----
        # walrus requires operands of an fp32r matmul to come from an
        # instruction that rounds/encodes to fp32r, so each DMA'd f32
        # staging tile is converted into an f32r tile by DVE (the Aw
        # build's tensor_mul does the conversion for free).
        xs = []
        xraw = []
        for k in range(8):
            xk_f = stg.tile([128, F_SLOTS], f32, tag="xstg")
            nc.sync.dma_start(out=xk_f[:], in_=x_d.ap()[128 * k:128 * (k + 1), :])
            xk = big.tile([128, F_SLOTS], f32r, tag=f"xs{k}")
            nc.vector.tensor_copy(xk[:], xk_f[:])
            xs.append(xk)
            xraw.append(xk_f)
        at = []
        for k in range(8):
            row = []
            for q in range(4):
                aq_f = stg.tile([128, 256], f32, tag="astg")
                nc.sync.dma_start(
                    out=aq_f[:],
                    in_=a_d.ap()[128 * k:128 * (k + 1), 256 * q:256 * (q + 1)])
                aq = big.tile([128, 256], f32r, tag=f"aw{k}_{q}")
                row.append((aq_f, aq))
            at.append(row)

        # ---- small setup chain ----
        w4 = sml.tile([4, 256], f32, tag="w4")
        nc.sync.dma_start(out=w4[:], in_=w_d.ap())
        asn = sml.tile([4, 256], f32, tag="asn")
        nc.sync.dma_start(out=asn[:], in_=c_d.ap()[0:4, :])
        one = sml.tile([4, 256], f32, tag="one")
        nc.sync.dma_start(out=one[:], in_=c_d.ap()[4:8, :])
        msk = sml.tile([1, 2], f32, tag="msk")
        nc.sync.dma_start(out=msk[:], in_=m_d.ap())
        w768 = sml.tile([1, 256], f32, tag="w768")
        nc.sync.dma_start(out=w768[:], in_=w_d.ap()[3:4, :])
        w0 = sml.tile([1, 256], f32, tag="w0")
        nc.sync.dma_start(out=w0[:], in_=w_d.ap()[0:1, :])

        with tc.tile_pool(name="psu", bufs=1, space="PSUM") as psu:
            # ws4[r] = sum_q w[256q+r] via ones-matmul (K=4, M=1)
            ps_ws = psu.tile([1, 256], f32, tag="ps_ws")
            nc.tensor.matmul(ps_ws[:], lhsT=one[0:4, 0:1], rhs=w4[:],
                             start=True, stop=True)
            ws4 = sml.tile([1, 256], f32, tag="ws4")
            nc.vector.tensor_copy(ws4[:], ps_ws[:])

            ws3a = sml.tile([1, 256], f32, tag="ws3a")
            nc.vector.tensor_sub(ws3a[:], ws4[:], w768[:])
            ws3b = sml.tile([1, 256], f32, tag="ws3b")
            nc.vector.tensor_sub(ws3b[:], ws4[:], w0[:])

            rws4 = sml.tile([1, 256], f32, tag="rws4")
            nc.vector.reciprocal(rws4[:], ws4[:])
            r3a = sml.tile([1, 256], f32, tag="r3a")
            nc.vector.reciprocal(r3a[:], ws3a[:])
            r3b = sml.tile([1, 256], f32, tag="r3b")
            nc.vector.reciprocal(r3b[:], ws3b[:])

            # fixup factors f = 1 + mask * (ws4/ws3x - 1)
            f0 = sml.tile([1, 256], f32, tag="f0")
            nc.vector.tensor_mul(f0[:], ws4[:], r3a[:])
            nc.vector.tensor_scalar_sub(f0[:], f0[:], 1.0)
            nc.vector.tensor_scalar_mul(f0[:], f0[:], msk[0:1, 0:1])
            nc.vector.tensor_scalar_add(f0[:], f0[:], 1.0)
            f7 = sml.tile([1, 256], f32, tag="f7")
            nc.vector.tensor_mul(f7[:], ws4[:], r3b[:])
            nc.vector.tensor_scalar_sub(f7[:], f7[:], 1.0)
            nc.vector.tensor_scalar_mul(f7[:], f7[:], msk[0:1, 1:2])
            nc.vector.tensor_scalar_add(f7[:], f7[:], 1.0)

            rfx0 = sml.tile([128, 256], f32, tag="rfx0")
            nc.vector.memset(rfx0[:], 1.0)
            nc.vector.tensor_copy(rfx0[0:1, :], f0[:])
            rfx7 = sml.tile([128, 256], f32, tag="rfx7")
            nc.vector.memset(rfx7[:], 1.0)
            nc.sync.dma_start(out=rfx7[126:127, :], in_=f7[:])

            # wnorm[q, r] = w[256q+r] * rws4[r]
            ps_b4 = psu.tile([4, 256], f32, tag="ps_b4")
            nc.tensor.matmul(ps_b4[:], lhsT=one[0:1, 0:4], rhs=rws4[:],
                             start=True, stop=True)
            wnorm = sml.tile([4, 256], f32, tag="wnorm")
            nc.vector.tensor_mul(wnorm[:], w4[:], ps_b4[:])

            # channel-1 taps (f32r so the matmul verifier is satisfied)
            tpu_f = sml.tile([4, 256], f32, tag="tpu_f")
            nc.scalar.mul(tpu_f[:], wnorm[:], 1.0 / N_FFT)
            tpu = sml.tile([4, 256], f32r, tag="tpu")
            nc.vector.tensor_copy(tpu[:], tpu_f[:])
            tpv = sml.tile([4, 256], f32r, tag="tpv")
            nc.vector.tensor_mul(tpv[:], tpu_f[:], asn[0:4, :])

            # broadcast wnorm rows to 128 partitions via DRAM bounce
            wnd = drm.tile([4, 256], f32, tag="wnd")
            nc.sync.dma_start(out=wnd[:], in_=wnorm[:])
            wbs = []
            for q in range(4):
                wb = sml.tile([128, 256], f32, tag=f"wb{q}")
                nc.sync.dma_start(
                    out=wb[:],
                    in_=wnd[q:q + 1, :].partition_broadcast(128)[:, 0, :])
                wbs.append(wb)

            # Aw_q = A_q * wnorm[q] (f32 staging in, f32r out)
            for k in range(8):
                for q in range(4):
                    aq_f, aq = at[k][q]
                    nc.vector.tensor_mul(aq[:], aq_f[:], wbs[q][:])

            # shifted zi[0], zi[512] rows for the imaginary channel
            tu_f = sml.tile([4, NB], f32, tag="tu_f")
            tv_f = sml.tile([4, NB], f32, tag="tv_f")
            for q in range(4):
                nc.sync.dma_start(out=tu_f[q:q + 1, :],
                                  in_=x_d.ap()[1024:1025, 3 - q:3 - q + NB])
                nc.sync.dma_start(out=tv_f[q:q + 1, :],
                                  in_=x_d.ap()[1025:1026, 3 - q:3 - q + NB])
            tu = sml.tile([4, NB], f32r, tag="tu")
            nc.vector.tensor_copy(tu[:], tu_f[:])
            tv = sml.tile([4, NB], f32r, tag="tv")
            nc.vector.tensor_copy(tv[:], tv_f[:])

            # ---- main matmul loop ----
            for tt in range(8):
                ps0 = pso.tile([128, 256], f32, tag="ps0")
                ps1 = pso.tile([128, 256], f32, tag="ps1")
                n_mm = 32
                i = 0
                for k in range(8):
                    for q in range(4):
                        off = tt * 128 + 3 - q
                        nc.tensor.matmul(
                            ps0[:],
                            lhsT=xs[k][:, off:off + 128],
                            rhs=at[k][q][1][:],
                            start=(i == 0), stop=(i == n_mm - 1))
                        i += 1
                nc.tensor.matmul(ps1[:],
                                 lhsT=tu[:, tt * 128:tt * 128 + 128],
                                 rhs=tpu[:],
                                 start=True, stop=False)
                nc.tensor.matmul(ps1[:],
                                 lhsT=tv[:, tt * 128:tt * 128 + 128],
                                 rhs=tpv[:],
                                 start=False, stop=True)

                o0 = osb.tile([128, 256], f32, tag="o0")
                o1 = osb.tile([128, 256], f32, tag="o1")
                if tt == 0:
                    nc.vector.tensor_mul(o0[:], ps0[:], rfx0[:])
                    nc.vector.tensor_mul(o1[:], ps1[:], rfx0[:])
                elif tt == 7:
                    nc.vector.tensor_mul(o0[:], ps0[:], rfx7[:])
                    nc.vector.tensor_mul(o1[:], ps1[:], rfx7[:])
                else:
                    nc.vector.tensor_copy(o0[:], ps0[:])
                    nc.vector.tensor_copy(o1[:], ps1[:])
                nc.sync.dma_start(
                    out=o_d.ap()[0:1, tt * 128:(tt + 1) * 128, :], in_=o0[:])
                nc.sync.dma_start(
                    out=o_d.ap()[1:2, tt * 128:(tt + 1) * 128, :], in_=o1[:])

    nc.compile()
    return nc


def _inputs_for_cores(z: np.ndarray, window: np.ndarray):
    amat = _CACHE.get("amat")
    if amat is None:
        amat = _amat()
        _CACHE["amat"] = amat
    consts = _CACHE.get("consts")
    if consts is None:
        consts = _consts()
        _CACHE["consts"] = consts
    w4 = np.ascontiguousarray(window.reshape(4, 256).astype(np.float32))

    in_maps = []
    for c in range(N_CORES):
        G = 1024 * c - 1  # global frame index of slot 0
        X = np.zeros((1026, F_SLOTS), np.float32)
        lo, hi = max(0, G), min(T_FRAMES, G + F_SLOTS)
        s0, s1 = lo - G, hi - G
        X[0:513, s0:s1] = z[0, :, lo:hi]
        X[513:1024, s0:s1] = z[1, 1:512, lo:hi]
        X[1024, s0:s1] = z[1, 0, lo:hi]
        X[1025, s0:s1] = z[1, 512, lo:hi]
        masks = np.array([[1.0 if c == 0 else 0.0,
                           1.0 if c == N_CORES - 1 else 0.0]], np.float32)
        in_maps.append({
            "x": X,
            "amat": amat,
            "wvec": w4,
            "consts": consts,
            "masks": masks,
        })
    return in_maps


def kernel(z: np.ndarray, window: np.ndarray) -> np.ndarray:
    from concourse.bass_utils import run_bass_kernel_spmd

    z = np.asarray(z, dtype=np.float32)
    window = np.asarray(window, dtype=np.float32)

    nc = _CACHE.get("nc")
    if nc is None:
        nc = _build_nc()
        _CACHE["nc"] = nc

    in_maps = _inputs_for_cores(z, window)
    res = run_bass_kernel_spmd(nc, in_maps, list(range(N_CORES)))

    parts = []
    for c in range(N_CORES):
        nb = NB if c < N_CORES - 1 else NB - 1
        o = res.results[c]["out"]  # [2, NB, 256]
        parts.append(o[:, :nb, :].reshape(2, -1))
    return np.ascontiguousarray(np.concatenate(parts, axis=1))


# revision 11
# speedup vs baseline: 1.4190x; 1.4190x over previous
"""Distributed ISTFT kernel for Trainium2 (8 NeuronCores, Bass/Tile).

Math (matches the jax reference):
  z: [2, 513, T] one-sided spectrum (real/imag), T = 8192 frames.
  Hermitian extension + ifft(1024) + window + overlap-add (hop 256) +
  divide by overlapped window sum + trim 512 each side -> [2, 2096896].

Key folds used here:
  * real(ifft) = A^T @ X where A [1024(k), 1024(n)] packs the cos rows for
    zr bins 0..512 and sin rows for zi bins 1..511; X packs those z rows.
  * imag(ifft)[n, t] = (zi[0,t] + (-1)^n zi[512,t]) / N  (rank-2).
  * Output sample m = 256*b + r; block b = sum_{q=0..3} wf_{b-q}[256q+r].
    Folding window * A and the reciprocal window-sum into the stationary
    operand gives O^T[t, r] = sum_q X[:, t+3-q]^T @ Aw_q directly -- the
    overlap-add, windowing and normalization all ride inside the matmul.
  * Frame axis is sharded 1024 output blocks/core with a 3-frame input
    halo, so no cross-core communication is needed at all.  The two
    blocks whose window-sum misses a frame (global block 2 and 8192) get
    a data-driven row fixup (masks make the same program a no-op on the
    other cores).
"""

import numpy as np

N_FFT = 1024
HOP = 256
T_FRAMES = 8192
N_CORES = 8
F_SLOTS = 1027  # frame slots per core: 1024 owned blocks need slots t..t+3
NB = 1024       # output blocks computed per core (core 7 uses 1023)

_CACHE = {}


def _amat() -> np.ndarray:
    """A [1024(kappa), 1024(n)]: ifft cos/sin weights, f32."""
    n = np.arange(N_FFT, dtype=np.float64)[None, :]
    k = np.arange(513, dtype=np.float64)[:, None]
    g = np.full((513, 1), 2.0)
    g[0, 0] = 1.0
    g[512, 0] = 1.0
    C = (g / N_FFT) * np.cos(2.0 * np.pi * k * n / N_FFT)
    k2 = np.arange(1, 512, dtype=np.float64)[:, None]
    S = (-2.0 / N_FFT) * np.sin(2.0 * np.pi * k2 * n / N_FFT)
    return np.ascontiguousarray(np.concatenate([C, S], 0).astype(np.float32))


def _consts() -> np.ndarray:
    c = np.zeros((8, 256), np.float32)
    c[0:4, :] = (1.0 - 2.0 * (np.arange(256) % 2)).astype(np.float32)[None, :]
    c[4:8, :] = 1.0
    return c


def _build_nc():
    from contextlib import ExitStack

    import concourse.tile as tile
    from concourse import bacc, mybir

    f32 = mybir.dt.float32
    f32r = mybir.dt.float32r

    nc = bacc.Bacc("TRN2", target_bir_lowering=False, debug=False,
                   num_devices=N_CORES)

    # x is pre-encoded to fp32r on the host (fp32 with the mantissa
    # rounded to 11 bits -- the PE's fp32r operand precision), so its DMA
    # is a valid fp32r producer and no on-device conversion is needed.
    x_d = nc.dram_tensor("x", [1026, F_SLOTS], f32r, kind="ExternalInput")
    a_d = nc.dram_tensor("amat", [1024, 1024], f32, kind="ExternalInput")
    w_d = nc.dram_tensor("wvec", [4, 256], f32, kind="ExternalInput")
    c_d = nc.dram_tensor("consts", [8, 256], f32, kind="ExternalInput")
    m_d = nc.dram_tensor("masks", [1, 2], f32, kind="ExternalInput")
    o_d = nc.dram_tensor("out", [2, NB, 256], f32, kind="ExternalOutput")

    with tile.TileContext(nc) as tc, ExitStack() as ctx:
        big = ctx.enter_context(tc.tile_pool(name="big", bufs=1))
        stg = ctx.enter_context(tc.tile_pool(name="stg", bufs=3))
        sml = ctx.enter_context(tc.tile_pool(name="sml", bufs=1))
        pso = ctx.enter_context(tc.tile_pool(name="pso", bufs=3, space="PSUM"))
        osb = ctx.enter_context(tc.tile_pool(name="osb", bufs=4))
        drm = ctx.enter_context(tc.tile_pool(name="drm", bufs=1, space="DRAM"))

        # ---- small setup inputs first: the window-sum chain gates the
        # Aw build, so get it going while the big DMAs stream ----
        w4 = sml.tile([4, 256], f32, tag="w4")
        nc.sync.dma_start(out=w4[:], in_=w_d.ap())
        asn = sml.tile([4, 256], f32, tag="asn")
        nc.sync.dma_start(out=asn[:], in_=c_d.ap()[0:4, :])
        one = sml.tile([4, 256], f32, tag="one")
        nc.sync.dma_start(out=one[:], in_=c_d.ap()[4:8, :])
        msk = sml.tile([1, 2], f32, tag="msk")
        nc.sync.dma_start(out=msk[:], in_=m_d.ap())
        w768 = sml.tile([1, 256], f32, tag="w768")
        nc.sync.dma_start(out=w768[:], in_=w_d.ap()[3:4, :])
        w0 = sml.tile([1, 256], f32, tag="w0")
        nc.sync.dma_start(out=w0[:], in_=w_d.ap()[0:1, :])

        # shifted zi[0], zi[512] rows (already fp32r-encoded in DRAM)
        tu = sml.tile([4, NB], f32r, tag="tu")
        tv = sml.tile([4, NB], f32r, tag="tv")
        for q in range(4):
            nc.sync.dma_start(out=tu[q:q + 1, :],
                              in_=x_d.ap()[1024:1025, 3 - q:3 - q + NB])
            nc.sync.dma_start(out=tv[q:q + 1, :],
                              in_=x_d.ap()[1025:1026, 3 - q:3 - q + NB])

        # ---- big loads: X chunks (f32r, used directly) and A staging ----
        xs = []
        at = []
        for k in range(8):
            xk = big.tile([128, F_SLOTS], f32r, tag=f"xs{k}")
            nc.sync.dma_start(out=xk[:], in_=x_d.ap()[128 * k:128 * (k + 1), :])
            xs.append(xk)
            ak_f = stg.tile([128, 1024], f32, tag="astg")
            nc.sync.dma_start(out=ak_f[:], in_=a_d.ap()[128 * k:128 * (k + 1), :])
            row = []
            for q in range(4):
                aq = big.tile([128, 256], f32r, tag=f"aw{k}_{q}")
                row.append((ak_f, aq))
            at.append(row)

        with tc.tile_pool(name="psu", bufs=1, space="PSUM") as psu:
            # ws4[r] = sum_q w[256q+r] via ones-matmul (K=4, M=1)
            ps_ws = psu.tile([1, 256], f32, tag="ps_ws")
            nc.tensor.matmul(ps_ws[:], lhsT=one[0:4, 0:1], rhs=w4[:],
                             start=True, stop=True)
            ws4 = sml.tile([1, 256], f32, tag="ws4")
            nc.vector.tensor_copy(ws4[:], ps_ws[:])

            ws3a = sml.tile([1, 256], f32, tag="ws3a")
            nc.vector.tensor_sub(ws3a[:], ws4[:], w768[:])
            ws3b = sml.tile([1, 256], f32, tag="ws3b")
            nc.vector.tensor_sub(ws3b[:], ws4[:], w0[:])

            rws4 = sml.tile([1, 256], f32, tag="rws4")
            nc.vector.reciprocal(rws4[:], ws4[:])
            r3a = sml.tile([1, 256], f32, tag="r3a")
            nc.vector.reciprocal(r3a[:], ws3a[:])
            r3b = sml.tile([1, 256], f32, tag="r3b")
            nc.vector.reciprocal(r3b[:], ws3b[:])

            # fixup factors f = 1 + mask * (ws4/ws3x - 1)
            f0 = sml.tile([1, 256], f32, tag="f0")
            nc.vector.tensor_mul(f0[:], ws4[:], r3a[:])
            nc.vector.tensor_scalar_sub(f0[:], f0[:], 1.0)
            nc.vector.tensor_scalar_mul(f0[:], f0[:], msk[0:1, 0:1])
            nc.vector.tensor_scalar_add(f0[:], f0[:], 1.0)
            f7 = sml.tile([1, 256], f32, tag="f7")
            nc.vector.tensor_mul(f7[:], ws4[:], r3b[:])
            nc.vector.tensor_scalar_sub(f7[:], f7[:], 1.0)
            nc.vector.tensor_scalar_mul(f7[:], f7[:], msk[0:1, 1:2])
            nc.vector.tensor_scalar_add(f7[:], f7[:], 1.0)

            rfx0 = sml.tile([128, 256], f32, tag="rfx0")
            nc.vector.memset(rfx0[:], 1.0)
            nc.vector.tensor_copy(rfx0[0:1, :], f0[:])
            rfx7 = sml.tile([128, 256], f32, tag="rfx7")
            nc.vector.memset(rfx7[:], 1.0)
            nc.sync.dma_start(out=rfx7[126:127, :], in_=f7[:])

            # wnorm[q, r] = w[256q+r] * rws4[r]
            ps_b4 = psu.tile([4, 256], f32, tag="ps_b4")
            nc.tensor.matmul(ps_b4[:], lhsT=one[0:1, 0:4], rhs=rws4[:],
                             start=True, stop=True)
            wnorm = sml.tile([4, 256], f32, tag="wnorm")
            nc.vector.tensor_mul(wnorm[:], w4[:], ps_b4[:])

            # channel-1 taps (f32r so the matmul verifier is satisfied)
            tpu_f = sml.tile([4, 256], f32, tag="tpu_f")
            nc.vector.tensor_scalar_mul(tpu_f[:], wnorm[:], 1.0 / N_FFT)
            tpu = sml.tile([4, 256], f32r, tag="tpu")
            nc.vector.tensor_copy(tpu[:], tpu_f[:])
            tpv = sml.tile([4, 256], f32r, tag="tpv")
            nc.vector.tensor_mul(tpv[:], tpu_f[:], asn[0:4, :])

            # broadcast wnorm rows to 128 partitions via DRAM bounce
            wnd = drm.tile([4, 256], f32, tag="wnd")
            nc.sync.dma_start(out=wnd[:], in_=wnorm[:])
            wbs = []
            for q in range(4):
                wb = sml.tile([128, 256], f32, tag=f"wb{q}")
                nc.sync.dma_start(
                    out=wb[:],
                    in_=wnd[q:q + 1, :].partition_broadcast(128)[:, 0, :])
                wbs.append(wb)

            # Aw_q = A[:, 256q:256q+256] * wnorm[q] (f32 staging in, f32r out)
            for k in range(8):
                for q in range(4):
                    ak_f, aq = at[k][q]
                    nc.vector.tensor_mul(
                        aq[:], ak_f[:, 256 * q:256 * (q + 1)], wbs[q][:])

            # ---- main matmul loop ----
            for tt in range(8):
                ps0 = pso.tile([128, 256], f32, tag="ps0")
                ps1 = pso.tile([128, 256], f32, tag="ps1")
                n_mm = 32
                i = 0
                for k in range(8):
                    for q in range(4):
                        off = tt * 128 + 3 - q
                        nc.tensor.matmul(
                            ps0[:],
                            lhsT=xs[k][:, off:off + 128],
                            rhs=at[k][q][1][:],
                            start=(i == 0), stop=(i == n_mm - 1))
                        i += 1
                nc.tensor.matmul(ps1[:],
                                 lhsT=tu[:, tt * 128:tt * 128 + 128],
                                 rhs=tpu[:],
                                 start=True, stop=False)
                nc.tensor.matmul(ps1[:],
                                 lhsT=tv[:, tt * 128:tt * 128 + 128],
                                 rhs=tpv[:],
                                 start=False, stop=True)

                o0 = osb.tile([128, 256], f32, tag="o0")
                o1 = osb.tile([128, 256], f32, tag="o1")
                if tt == 0:
                    nc.vector.tensor_mul(o0[:], ps0[:], rfx0[:])
                    nc.vector.tensor_mul(o1[:], ps1[:], rfx0[:])
                elif tt == 7:
                    nc.vector.tensor_mul(o0[:], ps0[:], rfx7[:])
                    nc.vector.tensor_mul(o1[:], ps1[:], rfx7[:])
                else:
                    nc.vector.tensor_copy(o0[:], ps0[:])
                    nc.vector.tensor_copy(o1[:], ps1[:])
                nc.sync.dma_start(
                    out=o_d.ap()[0:1, tt * 128:(tt + 1) * 128, :], in_=o0[:])
                nc.sync.dma_start(
                    out=o_d.ap()[1:2, tt * 128:(tt + 1) * 128, :], in_=o1[:])

    nc.compile()
    return nc


def _inputs_for_cores(z: np.ndarray, window: np.ndarray):
    amat = _CACHE.get("amat")
    if amat is None:
        amat = _amat()
        _CACHE["amat"] = amat
    consts = _CACHE.get("consts")
    if consts is None:
        consts = _consts()
        _CACHE["consts"] = consts
    w4 = np.ascontiguousarray(window.reshape(4, 256).astype(np.float32))

    in_maps = []
    for c in range(N_CORES):
        G = 1024 * c - 1  # global frame index of slot 0
        X = np.zeros((1026, F_SLOTS), np.float32)
        lo, hi = max(0, G), min(T_FRAMES, G + F_SLOTS)
        s0, s1 = lo - G, hi - G
        X[0:513, s0:s1] = z[0, :, lo:hi]
        X[513:1024, s0:s1] = z[1, 1:512, lo:hi]
        X[1024, s0:s1] = z[1, 0, lo:hi]
        X[1025, s0:s1] = z[1, 512, lo:hi]
        # pre-encode to fp32r: round the fp32 mantissa to 11 bits, which
        # is what the PE's fp32r operand path keeps
        u = X.view(np.uint32)
        u += np.uint32(0x800)
        u &= np.uint32(0xFFFFF000)
        masks = np.array([[1.0 if c == 0 else 0.0,
                           1.0 if c == N_CORES - 1 else 0.0]], np.float32)
        in_maps.append({
            "x": X,
            "amat": amat,
            "wvec": w4,
            "consts": consts,
            "masks": masks,
        })
    return in_maps


def kernel(z: np.ndarray, window: np.ndarray) -> np.ndarray:
    from concourse.bass_utils import run_bass_kernel_spmd

    z = np.asarray(z, dtype=np.float32)
    window = np.asarray(window, dtype=np.float32)

    nc = _CACHE.get("nc")
    if nc is None:
        nc = _build_nc()
        _CACHE["nc"] = nc

    in_maps = _inputs_for_cores(z, window)
    res = run_bass_kernel_spmd(nc, in_maps, list(range(N_CORES)))

    parts = []
    for c in range(N_CORES):
        nb = NB if c < N_CORES - 1 else NB - 1
        o = res.results[c]["out"]  # [2, NB, 256]
        parts.append(o[:, :nb, :].reshape(2, -1))
    return np.ascontiguousarray(np.concatenate(parts, axis=1))


# revision 16
# speedup vs baseline: 1.6168x; 1.1394x over previous
"""Distributed ISTFT kernel for Trainium2 (8 NeuronCores, Bass/Tile).

Math (matches the jax reference):
  z: [2, 513, T] one-sided spectrum (real/imag), T = 8192 frames.
  Hermitian extension + ifft(1024) + window + overlap-add (hop 256) +
  divide by overlapped window sum + trim 512 each side -> [2, 2096896].

Key folds used here:
  * real(ifft) = A^T @ X where A [1024(k), 1024(n)] packs the cos rows for
    zr bins 0..512 and sin rows for zi bins 1..511; X packs those z rows.
  * imag(ifft)[n, t] = (zi[0,t] + (-1)^n zi[512,t]) / N  (rank-2).
  * Output sample m = 256*b + r; block b = sum_{q=0..3} wf_{b-q}[256q+r].
    Folding window * A and the reciprocal window-sum into the stationary
    operand gives O^T[t, r] = sum_q X[:, t+3-q]^T @ Aw_q directly -- the
    overlap-add, windowing and normalization all ride inside the matmul.
  * Frame axis is sharded 1024 output blocks/core with a 3-frame input
    halo, so no cross-core communication is needed at all.  The two
    blocks whose window-sum misses a frame (global block 2 and 8192) get
    a data-driven row fixup (masks make the same program a no-op on the
    other cores).
"""

import numpy as np

N_FFT = 1024
HOP = 256
T_FRAMES = 8192
N_CORES = 8
F_SLOTS = 1027  # frame slots per core: 1024 owned blocks need slots t..t+3
NB = 1024       # output blocks computed per core (core 7 uses 1023)

_CACHE = {}


def _amat() -> np.ndarray:
    """A [1024(kappa), 1024(n)]: ifft cos/sin weights, f32."""
    n = np.arange(N_FFT, dtype=np.float64)[None, :]
    k = np.arange(513, dtype=np.float64)[:, None]
    g = np.full((513, 1), 2.0)
    g[0, 0] = 1.0
    g[512, 0] = 1.0
    C = (g / N_FFT) * np.cos(2.0 * np.pi * k * n / N_FFT)
    k2 = np.arange(1, 512, dtype=np.float64)[:, None]
    S = (-2.0 / N_FFT) * np.sin(2.0 * np.pi * k2 * n / N_FFT)
    return np.ascontiguousarray(np.concatenate([C, S], 0).astype(np.float32))


def _consts() -> np.ndarray:
    c = np.zeros((8, 256), np.float32)
    c[0:4, :] = (1.0 - 2.0 * (np.arange(256) % 2)).astype(np.float32)[None, :]
    c[4:8, :] = 1.0
    return c


def _build_nc():
    from contextlib import ExitStack

    import concourse.tile as tile
    from concourse import bacc, mybir

    f32 = mybir.dt.float32
    f32r = mybir.dt.float32r

    nc = bacc.Bacc("TRN2", target_bir_lowering=False, debug=False,
                   num_devices=N_CORES)

    # x is pre-encoded to fp32r on the host (fp32 with the mantissa
    # rounded to 11 bits -- the PE's fp32r operand precision), so its DMA
    # is a valid fp32r producer and no on-device conversion is needed.
    x_d = nc.dram_tensor("x", [1026, F_SLOTS], f32r, kind="ExternalInput")
    a_d = nc.dram_tensor("amat", [1024, 1024], f32, kind="ExternalInput")
    w_d = nc.dram_tensor("wvec", [1, N_FFT], f32, kind="ExternalInput")
    c_d = nc.dram_tensor("consts", [8, 256], f32, kind="ExternalInput")
    m_d = nc.dram_tensor("masks", [1, 2], f32, kind="ExternalInput")
    o_d = nc.dram_tensor("out", [2, NB, 256], f32, kind="ExternalOutput")

    with tile.TileContext(nc) as tc, ExitStack() as ctx:
        big = ctx.enter_context(tc.tile_pool(name="big", bufs=1))
        stg = ctx.enter_context(tc.tile_pool(name="stg", bufs=3))
        sml = ctx.enter_context(tc.tile_pool(name="sml", bufs=1))
        # bank budget: 5 (ps0) + 1 (psu) + 2 (ps1) = 8, opened in this
        # order so the regions never overlap and nothing waits
        ps0p = ctx.enter_context(tc.tile_pool(name="ps0p", bufs=5, space="PSUM"))
        psu = ctx.enter_context(tc.tile_pool(name="psu", bufs=1, space="PSUM"))
        ps1p = ctx.enter_context(tc.tile_pool(name="ps1p", bufs=2, space="PSUM"))
        osb = ctx.enter_context(tc.tile_pool(name="osb", bufs=4))
        drm = ctx.enter_context(tc.tile_pool(name="drm", bufs=1, space="DRAM"))

        # ---- tiny setup inputs first on the sync queue ----
        w4 = sml.tile([4, 256], f32, tag="w4")
        nc.sync.dma_start(out=w4[:],
                          in_=w_d.ap().rearrange("a (b c) -> (a b) c", c=256))
        asn = sml.tile([4, 256], f32, tag="asn")
        nc.sync.dma_start(out=asn[:], in_=c_d.ap()[0:4, :])
        one = sml.tile([4, 256], f32, tag="one")
        nc.sync.dma_start(out=one[:], in_=c_d.ap()[4:8, :])
        msk = sml.tile([1, 2], f32, tag="msk")
        nc.sync.dma_start(out=msk[:], in_=m_d.ap())
        w768 = sml.tile([1, 256], f32, tag="w768")
        nc.sync.dma_start(out=w768[:], in_=w_d.ap()[0:1, 768:1024])
        # raw-window broadcast [128, 1024] straight from the input: the Aw
        # build depends on nothing but this and the A chunks
        wbf = sml.tile([128, N_FFT], f32, tag="wbf")
        nc.sync.dma_start(
            out=wbf[:], in_=w_d.ap()[0:1, :].partition_broadcast(128)[:, 0, :])

        # ---- big loads: X chunks (f32r, used directly) + A staging,
        # with the Aw multiply trailing each A chunk ----
        xs = []
        aw = []
        for k in range(8):
            xk = big.tile([128, F_SLOTS], f32r, tag=f"xs{k}")
            nc.sync.dma_start(out=xk[:], in_=x_d.ap()[128 * k:128 * (k + 1), :])
            xs.append(xk)
            ak_f = stg.tile([128, N_FFT], f32, tag="astg")
            nc.sync.dma_start(out=ak_f[:], in_=a_d.ap()[128 * k:128 * (k + 1), :])
            awk = big.tile([128, N_FFT], f32r, tag=f"aw{k}")
            nc.vector.tensor_mul(awk[:], ak_f[:], wbf[:])
            aw.append(awk)
            if k == 3:
                # shifted zi[0], zi[512] rows (already fp32r in DRAM);
                # needed from ~24us, so queue them mid-stream
                tu = sml.tile([4, NB], f32r, tag="tu")
                tv = sml.tile([4, NB], f32r, tag="tv")
                for q in range(4):
                    nc.sync.dma_start(
                        out=tu[q:q + 1, :],
                        in_=x_d.ap()[1024:1025, 3 - q:3 - q + NB])
                    nc.sync.dma_start(
                        out=tv[q:q + 1, :],
                        in_=x_d.ap()[1025:1026, 3 - q:3 - q + NB])

        # ---- window-sum chain (gates evictions only, not matmuls) ----
        ps_ws = psu.tile([1, 256], f32, tag="ps_ws")
        nc.tensor.matmul(ps_ws[:], lhsT=one[0:4, 0:1], rhs=w4[:],
                         start=True, stop=True)
        rws4 = sml.tile([1, 256], f32, tag="rws4")
        nc.vector.reciprocal(rws4[:], ps_ws[:])
        rwsd = drm.tile([1, 256], f32, tag="rwsd")
        nc.gpsimd.dma_start(out=rwsd[:], in_=rws4[:])
        rwsb = sml.tile([128, 256], f32, tag="rwsb")
        nc.gpsimd.dma_start(
            out=rwsb[:], in_=rwsd[0:1, :].partition_broadcast(128)[:, 0, :])

        ws4 = sml.tile([1, 256], f32, tag="ws4")
        nc.vector.tensor_copy(ws4[:], ps_ws[:])
        ws3a = sml.tile([1, 256], f32, tag="ws3a")
        nc.vector.tensor_sub(ws3a[:], ws4[:], w768[:])
        ws3b = sml.tile([1, 256], f32, tag="ws3b")
        nc.vector.tensor_sub(ws3b[:], ws4[:], w4[0:1, :])
        r3a = sml.tile([1, 256], f32, tag="r3a")
        nc.vector.reciprocal(r3a[:], ws3a[:])
        r3b = sml.tile([1, 256], f32, tag="r3b")
        nc.vector.reciprocal(r3b[:], ws3b[:])

        # fixup factors f = 1 + mask * (ws4/ws3x - 1), blended into
        # full-height eviction-normalization tiles
        f0 = sml.tile([1, 256], f32, tag="f0")
        nc.vector.tensor_mul(f0[:], ws4[:], r3a[:])
        nc.vector.tensor_scalar_sub(f0[:], f0[:], 1.0)
        nc.vector.tensor_scalar_mul(f0[:], f0[:], msk[0:1, 0:1])
        nc.vector.tensor_scalar_add(f0[:], f0[:], 1.0)
        f7 = sml.tile([1, 256], f32, tag="f7")
        nc.vector.tensor_mul(f7[:], ws4[:], r3b[:])
        nc.vector.tensor_scalar_sub(f7[:], f7[:], 1.0)
        nc.vector.tensor_scalar_mul(f7[:], f7[:], msk[0:1, 1:2])
        nc.vector.tensor_scalar_add(f7[:], f7[:], 1.0)

        rfx0 = sml.tile([128, 256], f32, tag="rfx0")
        nc.vector.memset(rfx0[:], 1.0)
        nc.vector.tensor_copy(rfx0[0:1, :], f0[:])
        rfx7 = sml.tile([128, 256], f32, tag="rfx7")
        nc.vector.memset(rfx7[:], 1.0)
        nc.gpsimd.dma_start(out=rfx7[126:127, :], in_=f7[:])
        nrm0 = sml.tile([128, 256], f32, tag="nrm0")
        nc.vector.tensor_mul(nrm0[:], rwsb[:], rfx0[:])
        nrm7 = sml.tile([128, 256], f32, tag="nrm7")
        nc.vector.tensor_mul(nrm7[:], rwsb[:], rfx7[:])

        def norm_for(tt):
            return nrm0 if tt == 0 else (nrm7 if tt == 7 else rwsb)

        # channel-1 taps: raw window / N (the eviction applies 1/ws4 once)
        tpu = sml.tile([4, 256], f32r, tag="tpu")
        nc.vector.tensor_scalar_mul(tpu[:], w4[:], 1.0 / N_FFT)
        tpv_f = sml.tile([4, 256], f32, tag="tpv_f")
        nc.vector.tensor_mul(tpv_f[:], w4[:], asn[:])
        tpv = sml.tile([4, 256], f32r, tag="tpv")
        nc.vector.tensor_scalar_mul(tpv[:], tpv_f[:], 1.0 / N_FFT)

        def evict(ps, tt, ch):
            o = osb.tile([128, 256], f32, tag=f"o{ch}")
            nc.vector.tensor_mul(o[:], ps[:], norm_for(tt)[:])
            nc.scalar.dma_start(
                out=o_d.ap()[ch:ch + 1, tt * 128:(tt + 1) * 128, :], in_=o[:])

        # ---- channel 0: k-outer accumulation in two psum sweeps ----
        def sweep(tts):
            pss = {
                tt: ps0p.tile([128, 256], f32, tag="ps0", name=f"ps0_{tt}")
                for tt in tts
            }
            for k in range(8):
                for tt in tts:
                    for q in range(4):
                        off = tt * 128 + 3 - q
                        nc.tensor.matmul(
                            pss[tt][:],
                            lhsT=xs[k][:, off:off + 128],
                            rhs=aw[k][:, 256 * q:256 * (q + 1)],
                            start=(k == 0 and q == 0),
                            stop=(k == 7 and q == 3))
            for tt in tts:
                evict(pss[tt], tt, 0)

        sweep([0, 1, 2, 3, 4])

        # ---- channel 1 (tiny rank-2 matmuls) between the sweeps ----
        for tt in range(8):
            ps1 = ps1p.tile([128, 256], f32, tag="ps1")
            nc.tensor.matmul(ps1[:], lhsT=tu[:, tt * 128:tt * 128 + 128],
                             rhs=tpu[:], start=True, stop=False)
            nc.tensor.matmul(ps1[:], lhsT=tv[:, tt * 128:tt * 128 + 128],
                             rhs=tpv[:], start=False, stop=True)
            evict(ps1, tt, 1)

        sweep([5, 6, 7])

    nc.compile()
    return nc


def _inputs_for_cores(z: np.ndarray, window: np.ndarray):
    amat = _CACHE.get("amat")
    if amat is None:
        amat = _amat()
        _CACHE["amat"] = amat
    consts = _CACHE.get("consts")
    if consts is None:
        consts = _consts()
        _CACHE["consts"] = consts
    w4 = np.ascontiguousarray(window.reshape(1, 1024).astype(np.float32))

    in_maps = []
    for c in range(N_CORES):
        G = 1024 * c - 1  # global frame index of slot 0
        X = np.zeros((1026, F_SLOTS), np.float32)
        lo, hi = max(0, G), min(T_FRAMES, G + F_SLOTS)
        s0, s1 = lo - G, hi - G
        X[0:513, s0:s1] = z[0, :, lo:hi]
        X[513:1024, s0:s1] = z[1, 1:512, lo:hi]
        X[1024, s0:s1] = z[1, 0, lo:hi]
        X[1025, s0:s1] = z[1, 512, lo:hi]
        # pre-encode to fp32r: round the fp32 mantissa to 11 bits, which
        # is what the PE's fp32r operand path keeps
        u = X.view(np.uint32)
        u += np.uint32(0x800)
        u &= np.uint32(0xFFFFF000)
        masks = np.array([[1.0 if c == 0 else 0.0,
                           1.0 if c == N_CORES - 1 else 0.0]], np.float32)
        in_maps.append({
            "x": X,
            "amat": amat,
            "wvec": w4,
            "consts": consts,
            "masks": masks,
        })
    return in_maps


def kernel(z: np.ndarray, window: np.ndarray) -> np.ndarray:
    from concourse.bass_utils import run_bass_kernel_spmd

    z = np.asarray(z, dtype=np.float32)
    window = np.asarray(window, dtype=np.float32)

    nc = _CACHE.get("nc")
    if nc is None:
        nc = _build_nc()
        _CACHE["nc"] = nc

    in_maps = _inputs_for_cores(z, window)
    res = run_bass_kernel_spmd(nc, in_maps, list(range(N_CORES)))

    parts = []
    for c in range(N_CORES):
        nb = NB if c < N_CORES - 1 else NB - 1
        o = res.results[c]["out"]  # [2, NB, 256]
        parts.append(o[:, :nb, :].reshape(2, -1))
    return np.ascontiguousarray(np.concatenate(parts, axis=1))
